# revision 1
# baseline (speedup 1.0000x reference)
"""DeepGEMM-style fp8 linear on 8 TRN2 NeuronCores.

Computes: out = bf16( fp8(x_pad) @ (fp8(W) * block_scale).T ) + bias, sliced to
[16384, 4000], matching the jax reference (block scales are ones, bias zeros).

Strategy: batch-parallel SPMD. Each core gets a 2048-row batch shard of x
(transposed to [k, b] on host) plus the full weight (transposed to [k, n]
blocks on host). On device: quantize x and w to fp8_e4m3, fp8 matmul with
DoubleRow perf mode accumulating in fp32 PSUM, add bias + cast to bf16,
store out as [n, b]; host transposes/concats the shards back.

Batch-parallel beats the hinted column-parallel split here: replicating x
would move 268MB/core from HBM (~810us at 358GB/s); replicating w moves
66MB/core, total ~116MB/core (~320us), which hides under the ~500-870us of
TensorEngine work -> compute-bound instead of memory-bound.
"""

import sys

if "/opt/trn_rl_repo" not in sys.path:
    sys.path.insert(0, "/opt/trn_rl_repo")

import numpy as np
import ml_dtypes

P = 128
N_CORES = 8
BATCH = 16384
IN_F = 4000
OUT_F = 4000
K_PAD = 4096               # in-features padded to 32 k-subtiles of 128
N_PAD = 4096               # out-features padded 4032 -> 4096 (uniform n-tiles)

_kernel_cache = {}

# test.py knobs
TRACE = False
LAST_RESULTS = None


def _build(b_sh, ks, nt, bg, double_row, reps=1, kp_outer=True, probe="none",
           ramp_nt=0, xf_bufs=None, wq_bufs=2, wf_bufs=2, out_bufs=2,
           wq_engine="scalar", x_pair=False):
    import contextlib
    from concourse import bacc, tile, mybir
    from concourse.mybir import dt

    nbg = b_sh // bg
    assert nbg * bg == b_sh
    nc = bacc.Bacc(None, target_bir_lowering=False, debug=False)

    with tile.TileContext(nc) as tc:
        with tc.tile_pool(name="dram", bufs=1, space="DRAM") as dram:
            xt_shape = ([ks // 2, P, 2, b_sh] if x_pair else [ks, P, b_sh])
            xt = dram.tile(xt_shape, dt.float32, kind="ExternalInput",
                           name="xt", uniquify=False)
            wp = dram.tile([nt, P, ks, P], dt.float32, kind="ExternalInput",
                           name="wp", uniquify=False)
            bvec = dram.tile([P, nt], dt.bfloat16, kind="ExternalInput",
                             name="bvec", uniquify=False)
            out = dram.tile([nt, P, b_sh], dt.bfloat16, kind="ExternalOutput",
                            name="out", uniquify=False)

        with tc.tile_pool(name="const", bufs=1) as const, \
             tc.tile_pool(name="xqp", bufs=1) as xqp, \
             tc.tile_pool(name="xfp",
                          bufs=(xf_bufs if xf_bufs is not None
                                else (2 if ramp_nt else 3))) as xfp, \
             tc.tile_pool(name="wfp", bufs=wf_bufs) as wfp, \
             tc.tile_pool(name="wqp", bufs=wq_bufs) as wqp, \
             tc.tile_pool(name="wqr", bufs=max(ramp_nt, 1)) as wqr, \
             tc.tile_pool(name="prtp", bufs=max(ramp_nt, 1)) as prtp, \
             tc.tile_pool(name="outp", bufs=out_bufs) as outp, \
             tc.tile_pool(name="psp", bufs=(8 if bg <= 512 else 4),
                          space="PSUM") as psp, \
             (tc.For_i(0, reps, 1) if reps > 1
              else contextlib.nullcontext()):

            # bias: [P, nt] bf16 -> f32 (per-partition scalars, col = n-tile)
            bias_bf = const.tile([P, nt], dt.bfloat16)
            nc.sync.dma_start(out=bias_bf[:, :], in_=bvec[:, :])
            bias_sb = const.tile([P, nt], dt.float32)
            nc.vector.tensor_copy(bias_sb[:, :], bias_bf[:, :])

            # x: stream f32 k-subtiles, quantize to one resident fp8 tile
            xq = xqp.tile([P, ks, b_sh], dt.float8e4)
            if x_pair:
                # one DMA + quant per DoubleRow k-pair (16KB contiguous lines)
                for k2 in range(ks // 2):
                    xf = xfp.tile([P, 2, b_sh], dt.float32, name="xf")
                    nc.sync.dma_start(out=xf[:, :, :], in_=xt[k2])
                    nc.vector.tensor_copy(
                        xq[:, 2 * k2:2 * k2 + 2, :], xf[:, :, :])
            else:
                for k in range(ks):
                    xf = xfp.tile([P, b_sh], dt.float32, name="xf")
                    nc.sync.dma_start(out=xf[:, :], in_=xt[k])
                    nc.vector.tensor_copy(xq[:, k, :], xf[:, :])

            kk = ks // 2 if double_row else ks

            def load_w(n, pool):
                # weight n-tile: load f32 [P, ks, P] (contiguous
                # 16KB/partition), quantize to fp8. DMA on scalar's HWDGE
                # ring so w loads don't queue behind the x loads on sync's.
                wf = wfp.tile([P, ks, P], dt.float32, name="wf")
                nc.scalar.dma_start(out=wf[:, :, :], in_=wp[n])
                wq = pool.tile([P, ks, P], dt.float8e4, name="wq")
                if wq_engine == "scalar" or (wq_engine == "mixed" and n % 2):
                    nc.scalar.copy(wq[:, :, :], wf[:, :, :])
                else:
                    nc.vector.tensor_copy(wq[:, :, :], wf[:, :, :])
                return wq

            def mk_mm(wq):
                def mm(ps, g, k, start, stop):
                    if probe == "onetile":
                        k, g = 0, 0
                    if double_row:
                        nc.tensor.matmul(
                            ps[:, :],
                            lhsT=wq[:, 2 * k:2 * k + 2, :],
                            rhs=xq[:, 2 * k:2 * k + 2, g * bg:(g + 1) * bg],
                            start=start, stop=stop,
                            perf_mode=mybir.MatmulPerfMode.DoubleRow)
                    else:
                        nc.tensor.matmul(
                            ps[:, :],
                            lhsT=wq[:, k, :],
                            rhs=xq[:, k, g * bg:(g + 1) * bg],
                            start=start, stop=stop)
                return mm

            wq0 = None
            if probe == "peonly":
                # single weight tile loaded once: removes the 66MB w stream
                wq0 = load_w(0, wqp)

            # Ramp phase: while x is still streaming in, run the first-half-K
            # accumulation for the first ramp_nt n-tiles so the PE isn't
            # gated on the last k-subtile's arrival; partials park in SBUF.
            half = kk // 2
            ramp_wq, ramp_part = {}, {}
            for n in range(ramp_nt):
                rwq = load_w(n, wqr)
                ramp_wq[n] = rwq
                part = prtp.tile([P, b_sh], dt.float32, name="part")
                ramp_part[n] = part
                mm = mk_mm(rwq)
                pss = [psp.tile([P, bg], mybir.dt.float32, name="ps")
                       for _ in range(nbg)]
                for k in range(half):
                    for g in range(nbg):
                        mm(pss[g], g, k, k == 0, k == half - 1)
                for g in range(nbg):
                    nc.vector.tensor_copy(
                        part[:, g * bg:(g + 1) * bg], pss[g][:, :])

            for n in range(nt):
                ramp = n < ramp_nt
                if probe == "peonly":
                    wq = wq0
                elif ramp:
                    wq = ramp_wq[n]
                else:
                    wq = load_w(n, wqp)

                out_sb = outp.tile([P, b_sh], dt.bfloat16, name="out_sb")
                mm = mk_mm(wq)
                k_lo = half if ramp else 0

                def epilogue(g, ps):
                    dst = out_sb[:, g * bg:(g + 1) * bg]
                    if ramp:
                        # (psum + bias) + first-half partial -> bf16
                        nc.vector.scalar_tensor_tensor(
                            dst, ps[:, :], bias_sb[:, n:n + 1],
                            ramp_part[n][:, g * bg:(g + 1) * bg],
                            mybir.AluOpType.add, mybir.AluOpType.add)
                    else:
                        nc.vector.tensor_scalar_add(
                            dst, ps[:, :], bias_sb[:, n:n + 1])

                if probe == "noMM":
                    # one MM per psum tile: PE work ~1/16th, rest identical
                    pss = [psp.tile([P, bg], mybir.dt.float32, name="ps")
                           for _ in range(nbg)]
                    for g in range(nbg):
                        mm(pss[g], g, 0, True, True)
                    for g in range(nbg):
                        epilogue(g, pss[g])
                elif kp_outer:
                    # consecutive MMs share the stationary tile -> weight
                    # loads amortize/hide across nbg matmuls
                    pss = [psp.tile([P, bg], mybir.dt.float32, name="ps")
                           for _ in range(nbg)]
                    for k in range(k_lo, kk):
                        for g in range(nbg):
                            mm(pss[g], g, k, k == k_lo, k == kk - 1)
                    for g in range(nbg):
                        epilogue(g, pss[g])
                else:
                    for g in range(nbg):
                        ps = psp.tile([P, bg], mybir.dt.float32, name="ps")
                        for k in range(k_lo, kk):
                            mm(ps, g, k, k == k_lo, k == kk - 1)
                        epilogue(g, ps)

                nc.sync.dma_start(out=out[n], in_=out_sb[:, :])

    nc.finalize()
    return nc


def _get_nc(key):
    if key not in _kernel_cache:
        _kernel_cache[key] = _build(*key)
    return _kernel_cache[key]


def kernel(x, weight, weight_scale, bias):
    global LAST_RESULTS
    from concourse.bass_utils import run_bass_kernel_spmd

    x = np.asarray(x, dtype=np.float32)
    weight = np.asarray(weight, dtype=np.float32)
    weight_scale = np.asarray(weight_scale, dtype=np.float32)
    bias = np.asarray(bias)  # bf16

    n_out, k_pad = weight.shape          # 4032, 4096
    batch, in_f = x.shape                # 16384, 4000
    assert k_pad == K_PAD and batch == BATCH

    b_sh = batch // N_CORES
    ks = K_PAD // P
    nt = N_PAD // P
    bg = 512

    # weight_scale is ones per the module spec; fold it in best-effort if not.
    if not np.allclose(weight_scale, 1.0):
        ws = np.repeat(np.repeat(weight_scale, P, axis=0), P, axis=1)
        wq = weight.astype(ml_dtypes.float8_e4m3fn).astype(np.float32)
        weight = wq * ws[:n_out, :k_pad]

    # w -> [nt, p, ks, j]: element = w[nt*128 + j, ks*128 + p], zero-padded rows
    wpad = np.zeros((N_PAD, K_PAD), dtype=np.float32)
    wpad[:n_out] = weight
    wp = np.ascontiguousarray(
        wpad.reshape(nt, P, ks, P).transpose(0, 3, 2, 1))

    # bias -> [p, nt] bf16, zero-padded
    bpad = np.zeros(N_PAD, dtype=ml_dtypes.bfloat16)
    bpad[:n_out] = bias
    bvec = np.ascontiguousarray(bpad.reshape(nt, P).T)

    in_maps = []
    for c in range(N_CORES):
        shard = x[c * b_sh:(c + 1) * b_sh]          # [b_sh, in_f]
        xt = np.zeros((K_PAD, b_sh), dtype=np.float32)
        xt[:in_f] = shard.T
        in_maps.append({
            "xt": xt.reshape(ks, P, b_sh),
            "wp": wp,
            "bvec": bvec,
        })

    global _last_in_maps
    _last_in_maps = in_maps
    nc = _get_nc((b_sh, ks, nt, bg, True, 1))
    res = run_bass_kernel_spmd(nc, in_maps, list(range(N_CORES)), trace=TRACE)
    LAST_RESULTS = res

    final = np.empty((batch, OUT_F), dtype=ml_dtypes.bfloat16)
    for c in range(N_CORES):
        oc = res.results[c]["out"].reshape(N_PAD, b_sh)
        final[c * b_sh:(c + 1) * b_sh, :] = oc[:OUT_F].T
    return final



# revision 2
# speedup vs baseline: 1.3251x; 1.3251x over previous
"""DeepGEMM-style fp8 linear on 8 TRN2 NeuronCores.

Computes: out = bf16( fp8(x_pad) @ (fp8(W) * block_scale).T ) + bias, sliced to
[16384, 4000], matching the jax reference (block scales are ones, bias zeros).

Strategy: batch-parallel SPMD with HOST-side fp8 quantization. The reference
stores x and w as fp8_e4m3; quantizing on host (exactly reproducing the
reference's e4m3fn rounding — exact under TRN2's e4m3 container for this
data range) means each core streams 1-byte operands: x shard 8.4MB + full
weight 16.8MB + out 16.8MB ≈ 42MB/core, entirely hidden under the PE work.
On device: fp8 matmul (DoubleRow perf mode) accumulating f32 in PSUM,
bias-add + bf16 cast on DVE, store out as [n, b]; host transposes back.
"""

import sys

if "/opt/trn_rl_repo" not in sys.path:
    sys.path.insert(0, "/opt/trn_rl_repo")

import numpy as np
import ml_dtypes

P = 128
N_CORES = 8
BATCH = 16384
IN_F = 4000
OUT_F = 4000
K_PAD = 4096               # in-features padded to 32 k-subtiles of 128
N_PAD = 4096               # out-features padded 4032 -> 4096 (uniform n-tiles)

_kernel_cache = {}

# test.py knobs
TRACE = False
LAST_RESULTS = None


def _build(b_sh, ks, nt, bg, double_row, reps=1, ramp_nt=0, probe="none",
           wq_bufs=3, out_bufs=3, psum_bufs=8):
    import contextlib
    from concourse import bacc, tile, mybir
    from concourse.mybir import dt

    nbg = b_sh // bg
    assert nbg * bg == b_sh
    kk = ks // 2 if double_row else ks
    nc = bacc.Bacc(None, target_bir_lowering=False, debug=False)

    with tile.TileContext(nc) as tc:
        with tc.tile_pool(name="dram", bufs=1, space="DRAM") as dram:
            xt = dram.tile([ks // 2, P, 2, b_sh], dt.float8e4,
                           kind="ExternalInput", name="xt", uniquify=False)
            wp = dram.tile([nt, P, ks, P], dt.float8e4,
                           kind="ExternalInput", name="wp", uniquify=False)
            bvec = dram.tile([P, nt], dt.bfloat16, kind="ExternalInput",
                             name="bvec", uniquify=False)
            out = dram.tile([nt, P, b_sh], dt.bfloat16, kind="ExternalOutput",
                            name="out", uniquify=False)

        with tc.tile_pool(name="const", bufs=1) as const, \
             tc.tile_pool(name="xqp", bufs=1) as xqp, \
             tc.tile_pool(name="wqp", bufs=wq_bufs) as wqp, \
             tc.tile_pool(name="prtp", bufs=max(ramp_nt, 1)) as prtp, \
             tc.tile_pool(name="outp", bufs=out_bufs) as outp, \
             tc.tile_pool(name="psp", bufs=psum_bufs, space="PSUM") as psp, \
             (tc.For_i(0, reps, 1) if reps > 1
              else contextlib.nullcontext()):

            # bias: [P, nt] bf16 -> f32 (per-partition scalars, col = n-tile)
            bias_bf = const.tile([P, nt], dt.bfloat16)
            nc.sync.dma_start(out=bias_bf[:, :], in_=bvec[:, :])
            bias_sb = const.tile([P, nt], dt.float32)
            nc.vector.tensor_copy(bias_sb[:, :], bias_bf[:, :])

            # x: fp8 straight from HBM into one resident tile, k-pair per DMA
            # (4KB contiguous per partition line) on the sync HWDGE ring
            xq = xqp.tile([P, ks, b_sh], dt.float8e4)
            for k2 in range(ks // 2):
                nc.sync.dma_start(out=xq[:, 2 * k2:2 * k2 + 2, :], in_=xt[k2])

            def load_w(n, pool):
                # weight n-tile fp8 [P, ks, P]: 4KB contiguous per partition,
                # on scalar's HWDGE ring so w doesn't queue behind x/out
                wq = pool.tile([P, ks, P], dt.float8e4, name="wq")
                nc.scalar.dma_start(out=wq[:, :, :], in_=wp[n])
                return wq

            def mk_mm(wq):
                def mm(ps, g, k, start, stop):
                    if probe == "onetile":
                        k, g = 0, 0
                    if double_row:
                        nc.tensor.matmul(
                            ps[:, :],
                            lhsT=wq[:, 2 * k:2 * k + 2, :],
                            rhs=xq[:, 2 * k:2 * k + 2, g * bg:(g + 1) * bg],
                            start=start, stop=stop,
                            perf_mode=mybir.MatmulPerfMode.DoubleRow)
                    else:
                        nc.tensor.matmul(
                            ps[:, :],
                            lhsT=wq[:, k, :],
                            rhs=xq[:, k, g * bg:(g + 1) * bg],
                            start=start, stop=stop)
                return mm

            wq0 = None
            if probe == "peonly":
                # single weight tile loaded once: removes the w stream
                wq0 = load_w(0, wqp)

            # Ramp phase: while x is still streaming in, run the first-half-K
            # accumulation for the first ramp_nt n-tiles so the PE isn't
            # gated on the last k-subtile's arrival; partials park in SBUF.
            half = kk // 2
            ramp_wq, ramp_part = {}, {}
            for n in range(ramp_nt):
                rwq = load_w(n, wqp)
                ramp_wq[n] = rwq
                part = prtp.tile([P, b_sh], dt.float32, name="part")
                ramp_part[n] = part
                mm = mk_mm(rwq)
                pss = [psp.tile([P, bg], mybir.dt.float32, name="ps")
                       for _ in range(nbg)]
                for k in range(half):
                    for g in range(nbg):
                        mm(pss[g], g, k, k == 0, k == half - 1)
                for g in range(nbg):
                    nc.vector.tensor_copy(
                        part[:, g * bg:(g + 1) * bg], pss[g][:, :])

            for n in range(nt):
                ramp = n < ramp_nt
                if probe == "peonly":
                    wq = wq0
                elif ramp:
                    wq = ramp_wq[n]
                else:
                    wq = load_w(n, wqp)

                out_sb = outp.tile([P, b_sh], dt.bfloat16, name="out_sb")
                mm = mk_mm(wq)
                k_lo = half if ramp else 0

                def epilogue(g, ps):
                    dst = out_sb[:, g * bg:(g + 1) * bg]
                    if ramp:
                        # (psum + bias) + first-half partial -> bf16
                        nc.vector.scalar_tensor_tensor(
                            dst, ps[:, :], bias_sb[:, n:n + 1],
                            ramp_part[n][:, g * bg:(g + 1) * bg],
                            mybir.AluOpType.add, mybir.AluOpType.add)
                    else:
                        nc.vector.tensor_scalar_add(
                            dst, ps[:, :], bias_sb[:, n:n + 1])

                if probe == "noMM":
                    # one MM per psum tile: PE work ~1/16th, rest identical
                    pss = [psp.tile([P, bg], mybir.dt.float32, name="ps")
                           for _ in range(nbg)]
                    for g in range(nbg):
                        mm(pss[g], g, 0, True, True)
                    for g in range(nbg):
                        epilogue(g, pss[g])
                else:
                    # consecutive MMs share the stationary tile -> weight
                    # loads amortize/hide across nbg matmuls
                    pss = [psp.tile([P, bg], mybir.dt.float32, name="ps")
                           for _ in range(nbg)]
                    for k in range(k_lo, kk):
                        for g in range(nbg):
                            mm(pss[g], g, k, k == k_lo, k == kk - 1)
                    for g in range(nbg):
                        epilogue(g, pss[g])

                nc.sync.dma_start(out=out[n], in_=out_sb[:, :])

    nc.finalize()
    return nc


def _get_nc(key):
    if key not in _kernel_cache:
        _kernel_cache[key] = _build(*key)
    return _kernel_cache[key]


F8 = ml_dtypes.float8_e4m3          # TRN2's fp8e4 container
F8REF = ml_dtypes.float8_e4m3fn     # reference's quantization format


def kernel(x, weight, weight_scale, bias):
    global LAST_RESULTS
    from concourse.bass_utils import run_bass_kernel_spmd

    x = np.asarray(x, dtype=np.float32)
    weight = np.asarray(weight, dtype=np.float32)
    weight_scale = np.asarray(weight_scale, dtype=np.float32)
    bias = np.asarray(bias)  # bf16

    n_out, k_pad = weight.shape          # 4032, 4096
    batch, in_f = x.shape                # 16384, 4000
    assert k_pad == K_PAD and batch == BATCH

    b_sh = batch // N_CORES
    ks = K_PAD // P
    nt = N_PAD // P
    bg = 512

    # Quantize exactly like the reference (e4m3fn round-to-nearest), then
    # recast to the e4m3 container TRN2 uses — exact for values in range.
    wq8 = weight.astype(F8REF).astype(F8)
    if not np.allclose(weight_scale, 1.0):
        # best-effort fold of non-unit block scales (spec ships ones)
        ws = np.repeat(np.repeat(weight_scale, P, axis=0), P, axis=1)
        wdq = weight.astype(F8REF).astype(np.float32) * ws[:n_out, :k_pad]
        wq8 = wdq.astype(F8)

    # w -> [nt, p, ks, j]: element = w[nt*128 + j, ks*128 + p], zero-pad rows
    wpad = np.zeros((N_PAD, K_PAD), dtype=F8)
    wpad[:n_out] = wq8
    wp = np.ascontiguousarray(
        wpad.reshape(nt, P, ks, P).transpose(0, 3, 2, 1))

    # bias -> [p, nt] bf16, zero-padded
    bpad = np.zeros(N_PAD, dtype=ml_dtypes.bfloat16)
    bpad[:n_out] = bias
    bvec = np.ascontiguousarray(bpad.reshape(nt, P).T)

    xq8 = x.astype(F8REF).astype(F8)     # [batch, in_f] fp8
    in_maps = []
    for c in range(N_CORES):
        shard = xq8[c * b_sh:(c + 1) * b_sh]        # [b_sh, in_f]
        xt = np.zeros((K_PAD, b_sh), dtype=F8)
        xt[:in_f] = shard.T
        in_maps.append({
            "xt": xt.reshape(ks // 2, 2, P, b_sh).transpose(0, 2, 1, 3).copy(),
            "wp": wp,
            "bvec": bvec,
        })

    global _last_in_maps
    _last_in_maps = in_maps
    nc = _get_nc((b_sh, ks, nt, bg, True, 1))
    res = run_bass_kernel_spmd(nc, in_maps, list(range(N_CORES)), trace=TRACE)
    LAST_RESULTS = res

    final = np.empty((batch, OUT_F), dtype=ml_dtypes.bfloat16)
    for c in range(N_CORES):
        oc = res.results[c]["out"].reshape(N_PAD, b_sh)
        final[c * b_sh:(c + 1) * b_sh, :] = oc[:OUT_F].T
    return final
